# revision 1
# baseline (speedup 1.0000x reference)
"""Causal self-attention (B=4, T=2048, C=1024, H=16) on 8 trn2 NeuronCores.

Sharding: core c -> (batch b = c//2, head-group g = c%2 of 8 heads).
Each core computes its batch's QKV for its 8 heads, causal attention,
and a partial output projection (its heads' rows of w_out). Host sums
the two partials per batch and adds b_out.
"""
import sys
sys.path.insert(0, "/opt/trn_rl_repo")

import numpy as np
import concourse.bass as bass
import concourse.mybir as mybir
import concourse.tile as tile
from concourse import bacc
from concourse.bass_utils import run_bass_kernel_spmd
from concourse.tile import TileContext

F32 = mybir.dt.float32
F32R = mybir.dt.float32r
BF16 = mybir.dt.bfloat16
AF = mybir.ActivationFunctionType

B, T, C = 4, 2048, 1024
H, D = 16, 64
HL = 8            # heads per core
PAIRS = HL // 2   # head pairs (128-partition stacking)
KCH = C // 128    # contraction chunks for QKV
TG = T // 512     # 512-wide token groups
NKT = T // 128    # 128-wide key tiles
SCALE = D ** -0.5

_cache = {}


def _build(loop=1, phases=3):
    from contextlib import nullcontext
    nc = bacc.Bacc("TRN2", target_bir_lowering=False, debug=False, num_devices=8)

    xt_d = nc.dram_tensor("xt", [C, T], F32R, kind="ExternalInput")
    wqk_d = nc.dram_tensor("wqk", [C, 1024], F32R, kind="ExternalInput")
    wv_d = nc.dram_tensor("wv", [C + 1, 512], F32R, kind="ExternalInput")
    bqk_d = nc.dram_tensor("bqk", [128, 8], F32, kind="ExternalInput")
    wo_d = nc.dram_tensor("wo", [512, 1024], F32R, kind="ExternalInput")
    y_d = nc.dram_tensor("y", [T, C], F32, kind="ExternalOutput")

    with TileContext(nc) as tc:
        with tc.tile_pool(name="persist", bufs=1) as persist:
            loop_cm = tc.For_i(0, loop, 1) if loop > 1 else nullcontext()
            qkT = persist.tile([128, 8, T], F32R)          # tiles 0-3: q pairs, 4-7: k pairs
            v_aug = persist.tile([128, NKT, HL, D + 1], BF16)
            attout = persist.tile([128, PAIRS, T], F32R)
            bqk_sb = persist.tile([128, 8], F32)
            ones1 = persist.tile([1, 128], F32R)
            ones1_f = persist.tile([1, 128], F32)

            nc.sync.dma_start(out=bqk_sb, in_=bqk_d[:])
            nc.vector.memset(ones1_f, 1.0)
            nc.vector.tensor_copy(ones1, ones1_f)
            nc.vector.memset(v_aug[:, :, :, D:D + 1], 1.0)

            with loop_cm:
                # ---------------- Phase 1: QKV ----------------
                with tc.tile_pool(name="qkvw", bufs=1) as wpool, \
                     tc.tile_pool(name="xts", bufs=10) as xpool, \
                     tc.tile_pool(name="qk_ps", bufs=4, space="PSUM") as qk_psum, \
                     tc.tile_pool(name="v_ps", bufs=2, space="PSUM") as v_psum:
                    wqk_sb = wpool.tile([128, KCH, 1024], F32R)
                    wv_sb = wpool.tile([128, KCH, 512], F32R)
                    wv_last = wpool.tile([1, 512], F32R)
                    nc.sync.dma_start(
                        out=wqk_sb, in_=wqk_d.rearrange("(k p) c -> p k c", p=128))
                    nc.sync.dma_start(
                        out=wv_sb, in_=wv_d[0:C, :].rearrange("(k p) c -> p k c", p=128))
                    nc.sync.dma_start(out=wv_last, in_=wv_d[C:C + 1, :])

                    xt_r = xt_d.rearrange("(k p) t -> p k t", p=128)
                    for gi in range(TG):
                        xts = []
                        for k in range(KCH):
                            xk = xpool.tile([128, 512], F32R, tag="x")
                            nc.sync.dma_start(
                                out=xk, in_=xt_r[:, k, 512 * gi:512 * (gi + 1)])
                            xts.append(xk)
                        for t in range(8):
                            ps = qk_psum.tile([128, 512], F32)
                            for k in range(KCH):
                                nc.tensor.matmul(
                                    ps, wqk_sb[:, k, 128 * t:128 * (t + 1)], xts[k],
                                    start=(k == 0), stop=(k == KCH - 1))
                            nc.vector.tensor_scalar_add(
                                qkT[:, t, 512 * gi:512 * (gi + 1)], ps,
                                bqk_sb[:, t:t + 1])
                        for tt in range(4):
                            tau = 4 * gi + tt
                            ps = v_psum.tile([128, 512], F32)
                            for k in range(KCH):
                                nc.tensor.matmul(
                                    ps, xts[k][:, 128 * tt:128 * (tt + 1)],
                                    wv_sb[:, k, :], start=(k == 0), stop=False)
                            nc.tensor.matmul(
                                ps, ones1, wv_last, start=False, stop=True)
                            nc.vector.tensor_copy(
                                v_aug[:, tau, :, 0:D],
                                ps.rearrange("p (h d) -> p h d", h=HL))

                # ---------------- Phase 2: attention ----------------
                if phases < 3:
                    nc.gpsimd.dma_start(out=y_d[0:128, :], in_=qkT[:, 0, 0:1024])
                with tc.tile_pool(name="att", bufs=2) as att_pool, \
                     tc.tile_pool(name="nrm", bufs=2) as nrm_pool, \
                     tc.tile_pool(name="sc_ps", bufs=1, space="PSUM") as sc_psum, \
                     tc.tile_pool(name="av_ps", bufs=2, space="PSUM") as av_psum, \
                     tc.tile_pool(name="map_ps", bufs=1, space="PSUM") as map_psum:
                    def emit_scores(h, ki, atts):
                        p, r = h // 2, 64 * (h % 2)
                        q_t = qkT[r:r + 64, p, :]
                        k_t = qkT[r:r + 64, 4 + p, :]
                        qlo = 128 * ki
                        at = att_pool.tile([128, T - qlo], BF16, tag=f"att{ki}")
                        lhsT = k_t[:, qlo:qlo + 128]
                        sc = sc_psum.tile([128, T], F32)
                        qc = qlo
                        while qc < T:
                            qe = min(512 * (qc // 512 + 1), T)
                            nc.tensor.matmul(
                                sc[:, qc:qe], lhsT,
                                q_t[:, qc:qe], start=True, stop=True)
                            qc = qe
                        nc.scalar.activation(
                            at, sc[:, qlo:T], AF.Exp, scale=SCALE)
                        # zero strictly-upper part of the diagonal block
                        nc.gpsimd.affine_select(
                            out=at[:, 0:128], in_=at[:, 0:128],
                            compare_op=mybir.AluOpType.is_ge, fill=0.0,
                            base=0, pattern=[[1, 128]], channel_multiplier=-1)
                        atts.append(at)

                    def emit_av(h, gi, atts):
                        # AV: out_aug[65, q] accumulated over ki; row 64 = denom
                        p, r = h // 2, 64 * (h % 2)
                        av = av_psum.tile([128, 512], F32)
                        for ki in range(min(4 * gi + 4, NKT)):
                            qlo = 128 * ki
                            g0 = 512 * gi
                            lo = max(g0, qlo)
                            nc.tensor.matmul(
                                av[0:65, lo - g0:512],
                                v_aug[:, ki, h, :],
                                atts[ki][:, lo - qlo:512 * (gi + 1) - qlo],
                                start=(ki == 0), stop=(ki == min(4 * gi + 3, NKT - 1)))
                        # normalize: attout[r:r+64] = av[0:64] / bcast(av[64])
                        den = nrm_pool.tile([1, 512], F32R, tag="den")
                        nc.vector.tensor_copy(den, av[64:65, :])
                        mp = map_psum.tile([64, 512], F32)
                        nc.tensor.matmul(mp, ones1[:, 0:64], den,
                                         start=True, stop=True)
                        rmap = nrm_pool.tile([64, 512], F32, tag="rmap")
                        nc.vector.reciprocal_approx_fast(rmap, mp)
                        nc.vector.tensor_mul(
                            attout[r:r + 64, p, 512 * gi:512 * (gi + 1)],
                            av[0:64, :], rmap)

                    # software-pipelined emission: AV/normalize of head h-1
                    # interleaves between score groups of head h, keeping the
                    # PE instruction stream dense.
                    atts_prev = None
                    n_heads = HL if phases >= 2 else 0
                    for h in range(n_heads + (1 if n_heads else 0)):
                        atts_cur = []
                        for ki in range(NKT):
                            if h < n_heads:
                                emit_scores(h, ki, atts_cur)
                            if h >= 1 and ki % 4 == 3:
                                emit_av(h - 1, ki // 4, atts_prev)
                        atts_prev = atts_cur

                # ---------------- Phase 3: output projection ----------------
                with tc.tile_pool(name="proj", bufs=1) as wopool, \
                     tc.tile_pool(name="ysb", bufs=4) as ypool, \
                     tc.tile_pool(name="y_ps", bufs=4, space="PSUM") as y_psum:
                    wo_sb = wopool.tile([128, PAIRS, 1024], F32R)
                    if phases >= 3:
                        nc.gpsimd.dma_start(
                            out=wo_sb, in_=wo_d.rearrange("(p c) e -> c p e", c=128))
                    for tau in range(NKT if phases >= 3 else 0):
                        for eg in range(2):
                            ps = y_psum.tile([128, 512], F32)
                            for p in range(PAIRS):
                                nc.tensor.matmul(
                                    ps, attout[:, p, 128 * tau:128 * (tau + 1)],
                                    wo_sb[:, p, 512 * eg:512 * (eg + 1)],
                                    start=(p == 0), stop=(p == PAIRS - 1))
                            ysb = ypool.tile([128, 512], F32)
                            nc.vector.tensor_copy(ysb, ps)
                            nc.sync.dma_start(
                                out=y_d[128 * tau:128 * (tau + 1),
                                        512 * eg:512 * (eg + 1)],
                                in_=ysb)

    nc.compile()
    return nc


def _prep_inputs(x, w_qkv, b_qkv, w_out, b_out):
    x = np.asarray(x, np.float32)
    w_qkv = np.asarray(w_qkv, np.float32)
    b_qkv = np.asarray(b_qkv, np.float32)
    w_out = np.asarray(w_out, np.float32)
    in_maps = []
    for c in range(8):
        b, g = c // 2, c % 2
        xt = np.ascontiguousarray(x[b].T)
        wqk = np.concatenate(
            [w_qkv[:, 512 * g:512 * g + 512],
             w_qkv[:, C + 512 * g:C + 512 * g + 512]], axis=1)
        bqk = np.concatenate(
            [b_qkv[512 * g:512 * g + 512],
             b_qkv[C + 512 * g:C + 512 * g + 512]]).reshape(8, 128).T
        wv = np.concatenate(
            [w_qkv[:, 2 * C + 512 * g:2 * C + 512 * g + 512],
             b_qkv[None, 2 * C + 512 * g:2 * C + 512 * g + 512]], axis=0)
        wo = w_out[512 * g:512 * g + 512, :]
        in_maps.append({
            "xt": np.ascontiguousarray(xt),
            "wqk": np.ascontiguousarray(wqk),
            "bqk": np.ascontiguousarray(bqk),
            "wv": np.ascontiguousarray(wv),
            "wo": np.ascontiguousarray(wo),
        })
    return in_maps


def kernel(x, w_qkv, b_qkv, w_out, b_out):
    if "nc" not in _cache:
        _cache["nc"] = _build()
    nc = _cache["nc"]
    in_maps = _prep_inputs(x, w_qkv, b_qkv, w_out, b_out)
    res = run_bass_kernel_spmd(nc, in_maps, list(range(8)))
    b_out = np.asarray(b_out, np.float32)
    out = np.empty((B, T, C), np.float32)
    for b in range(B):
        out[b] = res.results[2 * b]["y"] + res.results[2 * b + 1]["y"] + b_out
    return out


def bench(x, w_qkv, b_qkv, w_out, b_out, iters=16, reps=3, loop=None, phases=3):
    """Time the NEFF on hardware. The kernel body is wrapped in a For_i
    hardware loop of `iters` iterations (one dispatch); subtracting the
    1-iteration dispatch time cancels network/dispatch overhead.
    Returns per-execution seconds."""
    import time
    import jax
    import jax.numpy as jnp
    from jax.sharding import Mesh, PartitionSpec
    from jax.experimental.shard_map import shard_map
    from concourse import bass2jax
    from concourse.bass2jax import (
        _bass_exec_p, install_neuronx_cc_hook, partition_id_tensor)

    if (loop is not None and loop > 1) or phases != 3:
        nc = _build(loop=loop or 1, phases=phases)
    else:
        nc = _cache.setdefault("nc", _build())
    install_neuronx_cc_hook()
    in_maps = _prep_inputs(x, w_qkv, b_qkv, w_out, b_out)

    partition_name = (nc.partition_id_tensor.name
                      if nc.partition_id_tensor else None)
    in_names, out_names, out_avals, zero_outs = [], [], [], []
    for alloc in nc.m.functions[0].allocations:
        if not isinstance(alloc, mybir.MemoryLocationSet):
            continue
        name = alloc.memorylocations[0].name
        if alloc.kind == "ExternalInput":
            if name != partition_name:
                in_names.append(name)
        elif alloc.kind == "ExternalOutput":
            out_names.append(name)
            shape = tuple(alloc.tensor_shape)
            dtype = mybir.dt.np(alloc.dtype)
            out_avals.append(jax.core.ShapedArray(shape, dtype))
            zero_outs.append(np.zeros(shape, dtype))
    n_params = len(in_names)
    all_names = in_names + out_names
    if partition_name is not None:
        all_names.append(partition_name)
    chain_idx = in_names.index("bqk")

    def body_n(n):
        def _body(*args):
            ins = list(args)
            outs = None
            for _ in range(n):
                cur = list(ins)
                if outs is not None:
                    y = outs[0]
                    cur[chain_idx] = cur[chain_idx] + 0.0 * y[:128, :8]
                if partition_name is not None:
                    cur.append(partition_id_tensor())
                outs = _bass_exec_p.bind(
                    *cur,
                    out_avals=tuple(out_avals),
                    in_names=tuple(all_names),
                    out_names=tuple(out_names),
                    lowering_input_output_aliases=(),
                    sim_require_finite=True,
                    sim_require_nnan=True,
                    nc=nc,
                )
            return tuple(outs)
        return _body

    devices = jax.devices()[:8]
    mesh = Mesh(np.asarray(devices), ("core",))
    in_specs = (PartitionSpec("core"),) * (n_params + len(out_names))
    out_specs = (PartitionSpec("core"),) * len(out_names)

    per_core = [[np.asarray(m[name]) for name in in_names] for m in in_maps]
    concat_in = [np.concatenate([per_core[c][i] for c in range(8)], axis=0)
                 for i in range(n_params)]
    concat_zero = [np.zeros((8 * z.shape[0], *z.shape[1:]), z.dtype)
                   for z in zero_outs]
    ins_dev = [jax.device_put(a) for a in concat_in]
    donate = tuple(range(n_params, n_params + len(zero_outs)))

    f = jax.jit(shard_map(body_n(1), mesh=mesh, in_specs=in_specs,
                          out_specs=out_specs, check_rep=False),
                donate_argnums=donate, keep_unused=True)

    def fresh_zeros(n):
        return [[jax.device_put(z) for z in concat_zero] for _ in range(n)]

    z0 = fresh_zeros(1)[0]
    jax.block_until_ready(f(*ins_dev, *z0))  # compile + warm

    def timed():
        best = float("inf")
        for _ in range(reps):
            zs = fresh_zeros(1)[0]
            jax.block_until_ready(zs)
            t0 = time.perf_counter()
            r = f(*ins_dev, *zs)
            jax.block_until_ready(r)
            best = min(best, time.perf_counter() - t0)
        return best

    return timed()



# revision 12
# speedup vs baseline: 3.2435x; 3.2435x over previous
"""Causal self-attention (B=4, T=2048, C=1024, H=16) on 8 trn2 NeuronCores.

Sharding: core c -> (batch b = c//2, head-group g = c%2 of 8 heads).
Each core computes its batch's QKV for its 8 heads, causal attention,
and a partial output projection (its heads' rows of w_out). Host sums
the two partials per batch and adds b_out.

v2 design (vs v1 baseline):
- Weights (wqk/wv/wo/bqk) are DMA'd into SBUF once, OUTSIDE the For_i
  loop: kills the ~20us PE stall at each iteration start.
- All matmuls in bf16 (x shipped as bf16): no fp32r small-free-dim
  penalty, half the DMA, half the SBUF.
- The gpsimd affine_select causal mask is replaced by a DVE multiply
  with a precomputed [128,128] mask tile.
- One fused PE instruction stream instead of 3 phases: V for all
  tokens + QK for head-pair 0 first, then 8 per-head "streams"
  (scores+exp for head h) with PE filler work interleaved under the
  ACT exp latency: AV matmuls of head h-1 and QK matmuls of the next
  head pair. Projection drains at the end, overlapped with AV of the
  last head. This keeps the PE dense so the HAM clock-gate stays at
  2.4 GHz (it throttles to 1.2 GHz when the PE has idle windows).
"""
import sys
sys.path.insert(0, "/opt/trn_rl_repo")

from collections import deque

import numpy as np
import ml_dtypes
import concourse.bass as bass
import concourse.mybir as mybir
import concourse.tile as tile
from concourse import bacc
from concourse.bass_utils import run_bass_kernel_spmd
from concourse.tile import TileContext

F32 = mybir.dt.float32
BF16 = mybir.dt.bfloat16
AF = mybir.ActivationFunctionType

B, T, C = 4, 2048, 1024
H, D = 16, 64
HL = 8            # heads per core
PAIRS = HL // 2   # head pairs (128-partition stacking)
KCH = C // 128    # contraction chunks for QKV
TG = T // 512     # 512-wide token groups
NKT = T // 128    # 128-wide key tiles
SCALE = D ** -0.5

_cache = {}


def _build(loop=1, phases=3):
    from contextlib import nullcontext
    nc = bacc.Bacc("TRN2", target_bir_lowering=False, debug=False, num_devices=8)

    xt_d = nc.dram_tensor("xt", [C, T], BF16, kind="ExternalInput")
    wqk_d = nc.dram_tensor("wqk", [C, 1024], BF16, kind="ExternalInput")
    wv_d = nc.dram_tensor("wv", [C + 1, 512], BF16, kind="ExternalInput")
    bqk_d = nc.dram_tensor("bqk", [128, 8], F32, kind="ExternalInput")
    wo_d = nc.dram_tensor("wo", [512, 1024], BF16, kind="ExternalInput")
    y_d = nc.dram_tensor("y", [T, C], BF16, kind="ExternalOutput")

    with TileContext(nc) as tc:
        with tc.tile_pool(name="persist", bufs=1) as persist:
            loop_cm = tc.For_i(0, loop, 1) if loop > 1 else nullcontext()
            wqk_sb = persist.tile([128, KCH, 1024], BF16)
            wo_sb = persist.tile([128, PAIRS, 1024], BF16)
            bqk_sb = persist.tile([128, 8], F32)
            xall = persist.tile([128, KCH, T], BF16)
            qkT = persist.tile([128, 8, T], BF16)       # t 0-3: q pairs, 4-7: k pairs
            v_aug = persist.tile([128, NKT, HL, D + 1], BF16)
            attout = persist.tile([128, PAIRS, T], BF16)
            mask = persist.tile([128, 128], BF16)       # causal: 1 if q>=k else 0
            ones_bf = persist.tile([1, 128], BF16)

            # weights + constants: once, outside the loop
            nc.sync.dma_start(out=bqk_sb, in_=bqk_d[:])
            nc.sync.dma_start(
                out=wqk_sb, in_=wqk_d.rearrange("(k p) c -> p k c", p=128))
            nc.sync.dma_start(
                out=wo_sb, in_=wo_d.rearrange("(p c) e -> c p e", c=128))
            nc.vector.memset(ones_bf, 1.0)
            nc.vector.memset(mask, 1.0)
            nc.gpsimd.affine_select(
                out=mask, in_=mask,
                compare_op=mybir.AluOpType.is_ge, fill=0.0,
                base=0, pattern=[[1, 128]], channel_multiplier=-1)
            nc.vector.memset(v_aug[:, :, :, D:D + 1], 1.0)

            xt_r = xt_d.rearrange("(k p) t -> p k t", p=128)

            with loop_cm:
                with tc.tile_pool(name="qk_ps", bufs=1, space="PSUM") as qk_psum, \
                     tc.tile_pool(name="av_ps", bufs=2, space="PSUM") as av_psum, \
                     tc.tile_pool(name="mp_ps", bufs=1, space="PSUM") as mp_psum, \
                     tc.tile_pool(name="att", bufs=2) as att_pool, \
                     tc.tile_pool(name="nrm", bufs=2) as nrm_pool:

                    # x for the whole iteration (per-group DMAs, sync queue)
                    for g in range(TG):
                        nc.sync.dma_start(
                            out=xall[:, :, 512 * g:512 * (g + 1)],
                            in_=xt_r[:, :, 512 * g:512 * (g + 1)])

                    def emit_qk(t, g):
                        # q or k feature tile t for 512-token group g
                        ps = qk_psum.tile([128, 512], F32, tag="qk")
                        for k in range(KCH):
                            nc.tensor.matmul(
                                ps, wqk_sb[:, k, 128 * t:128 * (t + 1)],
                                xall[:, k, 512 * g:512 * (g + 1)],
                                start=(k == 0), stop=(k == KCH - 1))
                        nc.vector.tensor_scalar_add(
                            qkT[:, t, 512 * g:512 * (g + 1)], ps,
                            bqk_sb[:, t:t + 1])

                    # ---- stage A: V for all tokens + QK for pair 0 ----
                    with tc.tile_pool(name="v_ps", bufs=2, space="PSUM") as v_psum, \
                         tc.tile_pool(name="wvp", bufs=1) as wv_pool:
                        wv_sb = wv_pool.tile([128, KCH, 512], BF16, tag="wv")
                        wv_last = wv_pool.tile([1, 512], BF16, tag="wvl")
                        nc.sync.dma_start(
                            out=wv_sb,
                            in_=wv_d[0:C, :].rearrange("(k p) c -> p k c", p=128))
                        nc.sync.dma_start(out=wv_last, in_=wv_d[C:C + 1, :])

                        def emit_v(g, tt):
                            tau = 4 * g + tt
                            ps = v_psum.tile([128, 512], F32, tag="v")
                            for k in range(KCH):
                                nc.tensor.matmul(
                                    ps, xall[:, k, 128 * tau:128 * (tau + 1)],
                                    wv_sb[:, k, :], start=(k == 0), stop=False)
                            nc.tensor.matmul(
                                ps, ones_bf, wv_last, start=False, stop=True)
                            nc.vector.tensor_copy(
                                v_aug[:, tau, :, 0:D],
                                ps.rearrange("p (h d) -> p h d", h=HL))

                        for g in range(TG):
                            emit_qk(0, g)
                            emit_v(g, 0)
                            emit_v(g, 1)
                            emit_qk(4, g)
                            emit_v(g, 2)
                            emit_v(g, 3)

                    # ---- filler units (run on PE under the ACT exp) ----
                    def emit_av(h, gi, ki, atts):
                        p, r = h // 2, 64 * (h % 2)
                        qlo = 128 * ki
                        g0 = 512 * gi
                        lo = max(g0, qlo)
                        av = _av_state.get((h, gi))
                        if av is None:
                            av = av_psum.tile([65, 512], F32, tag="av")
                            _av_state[(h, gi)] = av
                        nc.tensor.matmul(
                            av[:, lo - g0:512],
                            v_aug[:, ki, h, :],
                            atts[ki][:, lo - qlo:512 * (gi + 1) - qlo],
                            start=(ki == 0), stop=(ki == 4 * gi + 3))

                    def emit_den(h, gi):
                        av = _av_state[(h, gi)]
                        den = nrm_pool.tile([1, 512], BF16, tag="den")
                        nc.vector.tensor_copy(den, av[64:65, :])
                        _den_state[(h, gi)] = den

                    def emit_fin(h, gi):
                        p, r = h // 2, 64 * (h % 2)
                        av = _av_state.pop((h, gi))
                        den = _den_state.pop((h, gi))
                        mp = mp_psum.tile([64, 512], F32, tag="mp")
                        nc.tensor.matmul(mp, ones_bf[:, 0:64], den,
                                         start=True, stop=True)
                        rmap = nrm_pool.tile([64, 512], F32, tag="rmap")
                        nc.vector.reciprocal_approx_fast(rmap, mp)
                        nc.vector.tensor_mul(
                            attout[r:r + 64, p, 512 * gi:512 * (gi + 1)],
                            av[0:64, :], rmap)

                    _av_state = {}
                    _den_state = {}

                    UNIT_COST = {"av": 512, "den": 0, "fin": 512, "qk": 8 * 512}

                    def build_units(lag_h, lag_atts, qk_t):
                        units = []
                        if lag_h is not None:
                            for gi in range(TG):
                                for ki in range(4 * gi + 4):
                                    units.append(("av", lag_h, gi, ki, lag_atts))
                                units.append(("den", lag_h, gi))
                                units.append(("fin", lag_h, gi))
                        # stagger each fin 2 units later so the PE mp matmul
                        # doesn't stall on the DVE den copy
                        out = []
                        pending = deque()
                        for u in units:
                            if u[0] == "fin":
                                pending.append((len(out) + 2, u))
                            else:
                                out.append(u)
                            while pending and pending[0][0] <= len(out):
                                out.insert(pending[0][0], pending.popleft()[1])
                        while pending:
                            out.append(pending.popleft()[1])
                        if qk_t is not None:
                            # spread the 4 qk groups through the unit list
                            step = max(1, len(out) // 4)
                            for j in range(TG):
                                out.insert(min(len(out), (j + 1) * step + j),
                                           ("qk", qk_t, j))
                        return out

                    def run_unit(u):
                        if u[0] == "av":
                            emit_av(u[1], u[2], u[3], u[4])
                        elif u[0] == "den":
                            emit_den(u[1], u[2])
                        elif u[0] == "fin":
                            emit_fin(u[1], u[2])
                        elif u[0] == "qk":
                            emit_qk(u[1], u[2])

                    # ---- per-head streams ----
                    with tc.tile_pool(name="sc_ps", bufs=1, space="PSUM") as sc_psum:
                        atts_prev = None
                        qk_sched = [1, 5, 2, 6, 3, 7, None, None]
                        for h in range(HL):
                            p, r = h // 2, 64 * (h % 2)
                            q_t = qkT[r:r + 64, p, :]
                            k_t = qkT[r:r + 64, 4 + p, :]
                            units = deque(build_units(
                                h - 1 if h >= 1 else None, atts_prev,
                                qk_sched[h]))
                            total_cost = sum(UNIT_COST[u[0]] for u in units)
                            exp_total = sum(T - 128 * ki for ki in range(NKT))
                            atts_cur = []
                            for ki in range(NKT):
                                qlo = 128 * ki
                                w = T - qlo
                                sc = sc_psum.tile([128, T], F32, tag="sc")
                                lhsT = k_t[:, qlo:qlo + 128]
                                qc = qlo
                                while qc < T:
                                    # chunk ends on the 512-col PSUM bank grid:
                                    # a matmul output must not cross banks
                                    qe = min(512 * (qc // 512 + 1), T)
                                    nc.tensor.matmul(
                                        sc[:, qc:qe], lhsT, q_t[:, qc:qe],
                                        start=True, stop=True)
                                    qc = qe
                                at = att_pool.tile([128, w], BF16,
                                                   tag=f"att{ki}")
                                nc.scalar.activation(
                                    at, sc[:, qlo:T], AF.Exp, scale=SCALE)
                                nc.vector.tensor_mul(
                                    at[:, 0:128], at[:, 0:128], mask)
                                atts_cur.append(at)
                                # consume fillers proportional to exp width
                                budget = w * total_cost / exp_total
                                while units and budget > 0:
                                    u = units.popleft()
                                    run_unit(u)
                                    budget -= UNIT_COST[u[0]]
                            while units:
                                run_unit(units.popleft())
                            atts_prev = atts_cur

                    # ---- drain: AV of head 7 + projection ----
                    # ordering: av(gi) -> den(gi) -> first avs of gi+1 (PE
                    # filler while DVE copies den) -> fin(gi) -> proj(gi)
                    with tc.tile_pool(name="y_ps", bufs=2, space="PSUM") as y_psum, \
                         tc.tile_pool(name="ysb", bufs=3) as ypool:
                        h = HL - 1
                        done_av = 0
                        for gi in range(TG):
                            for ki in range(done_av, 4 * gi + 4):
                                emit_av(h, gi, ki, atts_prev)
                            emit_den(h, gi)
                            if gi + 1 < TG:
                                for ki in range(2):
                                    emit_av(h, gi + 1, ki, atts_prev)
                                done_av = 2
                            emit_fin(h, gi)
                            for tt in range(4):
                                tau = 4 * gi + tt
                                ps = y_psum.tile([128, 1024], F32, tag="y")
                                for pp in range(PAIRS):
                                    nc.tensor.matmul(
                                        ps[:, 0:512],
                                        attout[:, pp, 128 * tau:128 * (tau + 1)],
                                        wo_sb[:, pp, 0:512],
                                        start=(pp == 0), stop=(pp == PAIRS - 1))
                                for pp in range(PAIRS):
                                    nc.tensor.matmul(
                                        ps[:, 512:1024],
                                        attout[:, pp, 128 * tau:128 * (tau + 1)],
                                        wo_sb[:, pp, 512:1024],
                                        start=(pp == 0), stop=(pp == PAIRS - 1))
                                ysb = ypool.tile([128, 1024], BF16, tag="ysb")
                                nc.vector.tensor_copy(ysb, ps)
                                nc.gpsimd.dma_start(
                                    out=y_d[128 * tau:128 * (tau + 1), :],
                                    in_=ysb)

    nc.compile()
    return nc


def _prep_inputs(x, w_qkv, b_qkv, w_out, b_out):
    x = np.asarray(x, np.float32)
    w_qkv = np.asarray(w_qkv, np.float32)
    b_qkv = np.asarray(b_qkv, np.float32)
    w_out = np.asarray(w_out, np.float32)
    bf = ml_dtypes.bfloat16
    in_maps = []
    for c in range(8):
        b, g = c // 2, c % 2
        xt = np.ascontiguousarray(x[b].T)
        wqk = np.concatenate(
            [w_qkv[:, 512 * g:512 * g + 512],
             w_qkv[:, C + 512 * g:C + 512 * g + 512]], axis=1)
        bqk = np.concatenate(
            [b_qkv[512 * g:512 * g + 512],
             b_qkv[C + 512 * g:C + 512 * g + 512]]).reshape(8, 128).T
        wv = np.concatenate(
            [w_qkv[:, 2 * C + 512 * g:2 * C + 512 * g + 512],
             b_qkv[None, 2 * C + 512 * g:2 * C + 512 * g + 512]], axis=0)
        wo = w_out[512 * g:512 * g + 512, :]
        in_maps.append({
            "xt": np.ascontiguousarray(xt.astype(bf)),
            "wqk": np.ascontiguousarray(wqk.astype(bf)),
            "bqk": np.ascontiguousarray(bqk),
            "wv": np.ascontiguousarray(wv.astype(bf)),
            "wo": np.ascontiguousarray(wo.astype(bf)),
        })
    return in_maps


def kernel(x, w_qkv, b_qkv, w_out, b_out):
    if "nc" not in _cache:
        _cache["nc"] = _build()
    nc = _cache["nc"]
    in_maps = _prep_inputs(x, w_qkv, b_qkv, w_out, b_out)
    res = run_bass_kernel_spmd(nc, in_maps, list(range(8)))
    b_out = np.asarray(b_out, np.float32)
    out = np.empty((B, T, C), np.float32)
    for b in range(B):
        out[b] = (res.results[2 * b]["y"].astype(np.float32)
                  + res.results[2 * b + 1]["y"].astype(np.float32) + b_out)
    return out


def bench(x, w_qkv, b_qkv, w_out, b_out, iters=16, reps=3, loop=None, phases=3):
    """Time the NEFF on hardware. The kernel body is wrapped in a For_i
    hardware loop of `iters` iterations (one dispatch); subtracting the
    1-iteration dispatch time cancels network/dispatch overhead.
    Returns per-execution seconds."""
    import time
    import jax
    import jax.numpy as jnp
    from jax.sharding import Mesh, PartitionSpec
    from jax.experimental.shard_map import shard_map
    from concourse import bass2jax
    from concourse.bass2jax import (
        _bass_exec_p, install_neuronx_cc_hook, partition_id_tensor)

    if (loop is not None and loop > 1) or phases != 3:
        nc = _build(loop=loop or 1, phases=phases)
    else:
        nc = _cache.setdefault("nc", _build())
    install_neuronx_cc_hook()
    in_maps = _prep_inputs(x, w_qkv, b_qkv, w_out, b_out)

    partition_name = (nc.partition_id_tensor.name
                      if nc.partition_id_tensor else None)
    in_names, out_names, out_avals, zero_outs = [], [], [], []
    for alloc in nc.m.functions[0].allocations:
        if not isinstance(alloc, mybir.MemoryLocationSet):
            continue
        name = alloc.memorylocations[0].name
        if alloc.kind == "ExternalInput":
            if name != partition_name:
                in_names.append(name)
        elif alloc.kind == "ExternalOutput":
            out_names.append(name)
            shape = tuple(alloc.tensor_shape)
            dtype = mybir.dt.np(alloc.dtype)
            out_avals.append(jax.core.ShapedArray(shape, dtype))
            zero_outs.append(np.zeros(shape, dtype))
    n_params = len(in_names)
    all_names = in_names + out_names
    if partition_name is not None:
        all_names.append(partition_name)
    chain_idx = in_names.index("bqk")

    def body_n(n):
        def _body(*args):
            ins = list(args)
            outs = None
            for _ in range(n):
                cur = list(ins)
                if outs is not None:
                    y = outs[0]
                    cur[chain_idx] = cur[chain_idx] + 0.0 * y[:128, :8]
                if partition_name is not None:
                    cur.append(partition_id_tensor())
                outs = _bass_exec_p.bind(
                    *cur,
                    out_avals=tuple(out_avals),
                    in_names=tuple(all_names),
                    out_names=tuple(out_names),
                    lowering_input_output_aliases=(),
                    sim_require_finite=True,
                    sim_require_nnan=True,
                    nc=nc,
                )
            return tuple(outs)
        return _body

    devices = jax.devices()[:8]
    mesh = Mesh(np.asarray(devices), ("core",))
    in_specs = (PartitionSpec("core"),) * (n_params + len(out_names))
    out_specs = (PartitionSpec("core"),) * len(out_names)

    per_core = [[np.asarray(m[name]) for name in in_names] for m in in_maps]
    concat_in = [np.concatenate([per_core[c][i] for c in range(8)], axis=0)
                 for i in range(n_params)]
    concat_zero = [np.zeros((8 * z.shape[0], *z.shape[1:]), z.dtype)
                   for z in zero_outs]
    ins_dev = [jax.device_put(a) for a in concat_in]
    donate = tuple(range(n_params, n_params + len(zero_outs)))

    f = jax.jit(shard_map(body_n(1), mesh=mesh, in_specs=in_specs,
                          out_specs=out_specs, check_rep=False),
                donate_argnums=donate, keep_unused=True)

    def fresh_zeros(n):
        return [[jax.device_put(z) for z in concat_zero] for _ in range(n)]

    z0 = fresh_zeros(1)[0]
    jax.block_until_ready(f(*ins_dev, *z0))  # compile + warm

    def timed():
        best = float("inf")
        for _ in range(reps):
            zs = fresh_zeros(1)[0]
            jax.block_until_ready(zs)
            t0 = time.perf_counter()
            r = f(*ins_dev, *zs)
            jax.block_until_ready(r)
            best = min(best, time.perf_counter() - t0)
        return best

    return timed()


# revision 25
# speedup vs baseline: 3.6213x; 1.1165x over previous
"""Causal self-attention (B=4, T=2048, C=1024, H=16) on 8 trn2 NeuronCores.

Sharding: core c -> (batch b = c//2, head-group g = c%2 of 8 heads).
Each core computes its batch's QKV for its 8 heads, causal attention,
and a partial output projection (its heads' rows of w_out). Host sums
the two partials per batch and adds b_out.

v2 design (vs v1 baseline):
- Weights (wqk/wv/wo/bqk) are DMA'd into SBUF once, OUTSIDE the For_i
  loop: kills the ~20us PE stall at each iteration start.
- All matmuls in bf16 (x shipped as bf16): no fp32r small-free-dim
  penalty, half the DMA, half the SBUF.
- The gpsimd affine_select causal mask is replaced by a DVE multiply
  with a precomputed [128,128] mask tile.
- One fused PE instruction stream instead of 3 phases: V for all
  tokens + QK for head-pair 0 first, then 8 per-head "streams"
  (scores+exp for head h) with PE filler work interleaved under the
  ACT exp latency: AV matmuls of head h-1 and QK matmuls of the next
  head pair. Projection drains at the end, overlapped with AV of the
  last head. This keeps the PE dense so the HAM clock-gate stays at
  2.4 GHz (it throttles to 1.2 GHz when the PE has idle windows).
"""
import sys
sys.path.insert(0, "/opt/trn_rl_repo")

from collections import deque

import numpy as np
import ml_dtypes
import concourse.bass as bass
import concourse.mybir as mybir
import concourse.tile as tile
from concourse import bacc
from concourse.bass_utils import run_bass_kernel_spmd
from concourse.tile import TileContext

F32 = mybir.dt.float32
BF16 = mybir.dt.bfloat16
AF = mybir.ActivationFunctionType

B, T, C = 4, 2048, 1024
H, D = 16, 64
HL = 8            # heads per core
PAIRS = HL // 2   # head pairs (128-partition stacking)
KCH = C // 128    # contraction chunks for QKV
TG = T // 512     # 512-wide token groups
NKT = T // 128    # 128-wide key tiles
SCALE = D ** -0.5

_cache = {}


def _build(loop=1, phases=3):
    from contextlib import nullcontext
    nc = bacc.Bacc("TRN2", target_bir_lowering=False, debug=False, num_devices=8)

    xt_d = nc.dram_tensor("xt", [C, T], BF16, kind="ExternalInput")
    wqk_d = nc.dram_tensor("wqk", [C, 1024], BF16, kind="ExternalInput")
    wv_d = nc.dram_tensor("wv", [C, 512], BF16, kind="ExternalInput")
    bqk_d = nc.dram_tensor("bqk", [128, 8], F32, kind="ExternalInput")
    wo_d = nc.dram_tensor("wo", [512, 1024], BF16, kind="ExternalInput")
    y_d = nc.dram_tensor("y", [T, C], BF16, kind="ExternalOutput")

    with TileContext(nc) as tc:
        with tc.tile_pool(name="persist", bufs=1) as persist:
            loop_cm = tc.For_i(0, loop, 1) if loop > 1 else nullcontext()
            wqk_sb = persist.tile([128, KCH, 1024], BF16)
            wo_sb = persist.tile([128, PAIRS, 1024], BF16)
            bqk_sb = persist.tile([128, 8], F32)
            xall = persist.tile([128, KCH, T], BF16)
            qkT = persist.tile([128, 8, T], BF16)       # t 0-3: q pairs, 4-7: k pairs
            v_aug = persist.tile([128, NKT, HL, D + 1], BF16)
            attout = persist.tile([128, PAIRS, T], BF16)
            mask = persist.tile([128, 128], BF16)       # causal: 1 if q>=k else 0
            ones_bf = persist.tile([1, 128], BF16)

            # weights + constants: once, outside the loop
            nc.sync.dma_start(out=bqk_sb, in_=bqk_d[:])
            nc.sync.dma_start(
                out=wqk_sb, in_=wqk_d.rearrange("(k p) c -> p k c", p=128))
            nc.sync.dma_start(
                out=wo_sb, in_=wo_d.rearrange("(p c) e -> c p e", c=128))
            nc.vector.memset(ones_bf, 1.0)
            nc.vector.memset(mask, 1.0)
            nc.gpsimd.affine_select(
                out=mask, in_=mask,
                compare_op=mybir.AluOpType.is_ge, fill=0.0,
                base=0, pattern=[[1, 128]], channel_multiplier=-1)
            nc.vector.memset(v_aug[:, :, :, D:D + 1], 1.0)

            xt_r = xt_d.rearrange("(k p) t -> p k t", p=128)

            with loop_cm:
                with tc.tile_pool(name="qk_ps", bufs=2, space="PSUM") as qk_psum, \
                     tc.tile_pool(name="av_ps", bufs=2, space="PSUM") as av_psum, \
                     tc.tile_pool(name="att", bufs=2) as att_pool, \
                     tc.tile_pool(name="nrm", bufs=2) as nrm_pool:

                    # x for the whole iteration (per-group DMAs, sync queue)
                    for g in range(TG):
                        nc.sync.dma_start(
                            out=xall[:, :, 512 * g:512 * (g + 1)],
                            in_=xt_r[:, :, 512 * g:512 * (g + 1)])

                    def emit_qk(t, g):
                        # q or k feature tile t for 512-token group g
                        ps = qk_psum.tile([128, 512], F32, tag="qk")
                        for k in range(KCH):
                            nc.tensor.matmul(
                                ps, wqk_sb[:, k, 128 * t:128 * (t + 1)],
                                xall[:, k, 512 * g:512 * (g + 1)],
                                start=(k == 0), stop=(k == KCH - 1))
                        nc.vector.tensor_scalar_add(
                            qkT[:, t, 512 * g:512 * (g + 1)], ps,
                            bqk_sb[:, t:t + 1])

                    # ---- stage A: V for all tokens + QK for pair 0 ----
                    # V bias is folded into the host-side output bias
                    # (softmax weights sum to 1, so +b_v adds b_v @ w_out to y)
                    with tc.tile_pool(name="v_ps", bufs=2, space="PSUM") as v_psum, \
                         tc.tile_pool(name="wvp", bufs=1) as wv_pool:
                        wv_sb = wv_pool.tile([128, KCH, 512], BF16, tag="wv")
                        nc.sync.dma_start(
                            out=wv_sb,
                            in_=wv_d.rearrange("(k p) c -> p k c", p=128))

                        def emit_v(g, tt):
                            tau = 4 * g + tt
                            ps = v_psum.tile([128, 512], F32, tag="v")
                            for k in range(KCH):
                                nc.tensor.matmul(
                                    ps, xall[:, k, 128 * tau:128 * (tau + 1)],
                                    wv_sb[:, k, :], start=(k == 0),
                                    stop=(k == KCH - 1))
                            nc.vector.tensor_copy(
                                v_aug[:, tau, :, 0:D],
                                ps.rearrange("p (h d) -> p h d", h=HL))

                        for g in range(TG):
                            emit_qk(0, g)
                            emit_v(g, 0)
                            emit_v(g, 1)
                            emit_qk(4, g)
                            emit_v(g, 2)
                            emit_v(g, 3)

                    # ---- filler units (run on PE under the ACT exp) ----
                    def emit_av(h, gi, ki, atts):
                        p, r = h // 2, 64 * (h % 2)
                        qlo = 128 * ki
                        g0 = 512 * gi
                        lo = max(g0, qlo)
                        av = _av_state.get((h, gi))
                        if av is None:
                            av = av_psum.tile([65, 512], F32, tag="av")
                            _av_state[(h, gi)] = av
                        nc.tensor.matmul(
                            av[:, lo - g0:512],
                            v_aug[:, ki, h, :],
                            atts[ki][:, lo - qlo:512 * (gi + 1) - qlo],
                            start=(ki == 0), stop=(ki == 4 * gi + 3))

                    def emit_den(h, gi):
                        av = _av_state[(h, gi)]
                        den = nrm_pool.tile([1, 512], F32, tag="den")
                        nc.vector.tensor_copy(den, av[64:65, :])
                        nc.vector.reciprocal_approx_fast(den, den)
                        _den_state[(h, gi)] = den

                    def emit_fin(h, gi):
                        p, r = h // 2, 64 * (h % 2)
                        av = _av_state.pop((h, gi))
                        rden = _den_state.pop((h, gi))
                        rbc = nrm_pool.tile([64, 512], F32, tag="rbc")
                        nc.gpsimd.partition_broadcast(rbc, rden)
                        nc.vector.tensor_mul(
                            attout[r:r + 64, p, 512 * gi:512 * (gi + 1)],
                            av[0:64, :], rbc)

                    _av_state = {}
                    _den_state = {}

                    UNIT_COST = {"av": 512, "den": 64, "fin": 64, "qk": 8 * 512}

                    def build_units(lag_h, lag_atts, qk_ts):
                        units = []
                        if lag_h is not None:
                            for gi in range(TG):
                                for ki in range(4 * gi + 4):
                                    units.append(("av", lag_h, gi, ki, lag_atts))
                                units.append(("den", lag_h, gi))
                                units.append(("fin", lag_h, gi))
                        # stagger each fin 2 units later so the DVE mul does
                        # not wait back-to-back on the den/recip/bcast chain
                        out = []
                        pending = deque()
                        for u in units:
                            if u[0] == "fin":
                                pending.append((len(out) + 2, u))
                            else:
                                out.append(u)
                            while pending and pending[0][0] <= len(out):
                                out.insert(pending[0][0], pending.popleft()[1])
                        while pending:
                            out.append(pending.popleft()[1])
                        # spread the qk groups through the unit list
                        qk_units = [("qk", t, j) for t in qk_ts for j in range(TG)]
                        if qk_units:
                            n = len(qk_units)
                            step = max(1, len(out) // n)
                            for j, u in enumerate(qk_units):
                                out.insert(min(len(out), (j + 1) * step + j), u)
                        return out

                    def run_unit(u):
                        if u[0] == "av":
                            emit_av(u[1], u[2], u[3], u[4])
                        elif u[0] == "den":
                            emit_den(u[1], u[2])
                        elif u[0] == "fin":
                            emit_fin(u[1], u[2])
                        elif u[0] == "qk":
                            emit_qk(u[1], u[2])

                    # ---- per-head streams ----
                    with tc.tile_pool(name="sc_ps", bufs=1, space="PSUM") as sc_psum:
                        atts_prev = None
                        qk_sched = [[1, 5], [2], [6], [3], [7], [], [], []]
                        for h in range(HL):
                            p, r = h // 2, 64 * (h % 2)
                            q_t = qkT[r:r + 64, p, :]
                            k_t = qkT[r:r + 64, 4 + p, :]
                            units = deque(build_units(
                                h - 1 if h >= 1 else None, atts_prev,
                                qk_sched[h]))
                            total_cost = sum(UNIT_COST[u[0]] for u in units)
                            exp_total = sum(T - 128 * ki for ki in range(NKT))
                            atts_cur = []
                            for ki in range(NKT):
                                qlo = 128 * ki
                                w = T - qlo
                                sc = sc_psum.tile([128, T], F32, tag="sc")
                                lhsT = k_t[:, qlo:qlo + 128]
                                qc = qlo
                                while qc < T:
                                    # chunk ends on the 512-col PSUM bank grid:
                                    # a matmul output must not cross banks
                                    qe = min(512 * (qc // 512 + 1), T)
                                    nc.tensor.matmul(
                                        sc[:, qc:qe], lhsT, q_t[:, qc:qe],
                                        start=True, stop=True)
                                    qc = qe
                                at = att_pool.tile([128, w], BF16,
                                                   tag=f"att{ki}")
                                nc.scalar.activation(
                                    at, sc[:, qlo:T], AF.Exp, scale=SCALE)
                                nc.vector.tensor_mul(
                                    at[:, 0:128], at[:, 0:128], mask)
                                atts_cur.append(at)
                                # last head: AV for completed gi-groups runs
                                # in-stream (no next stream to lag into)
                                if h == HL - 1 and ki % 4 == 3 and ki < 12:
                                    gi = ki // 4
                                    for kj in range(4 * gi + 4):
                                        emit_av(h, gi, kj, atts_cur)
                                    emit_den(h, gi)
                                    if gi >= 1:
                                        emit_fin(h, gi - 1)
                                # consume fillers proportional to exp width
                                budget = w * total_cost / exp_total
                                while units and budget > 0:
                                    u = units.popleft()
                                    run_unit(u)
                                    budget -= UNIT_COST[u[0]]
                            while units:
                                run_unit(units.popleft())
                            atts_prev = atts_cur

                    # ---- drain: last AV group of head 7 + projection ----
                    with tc.tile_pool(name="y_ps", bufs=2, space="PSUM") as y_psum, \
                         tc.tile_pool(name="ysb", bufs=3) as ypool:
                        h = HL - 1

                        def emit_proj(gi):
                            for tt in range(4):
                                tau = 4 * gi + tt
                                ps = y_psum.tile([128, 1024], F32, tag="y")
                                for pp in range(PAIRS):
                                    nc.tensor.matmul(
                                        ps[:, 0:512],
                                        attout[:, pp, 128 * tau:128 * (tau + 1)],
                                        wo_sb[:, pp, 0:512],
                                        start=(pp == 0), stop=(pp == PAIRS - 1))
                                for pp in range(PAIRS):
                                    nc.tensor.matmul(
                                        ps[:, 512:1024],
                                        attout[:, pp, 128 * tau:128 * (tau + 1)],
                                        wo_sb[:, pp, 512:1024],
                                        start=(pp == 0), stop=(pp == PAIRS - 1))
                                ysb = ypool.tile([128, 1024], BF16, tag="ysb")
                                nc.vector.tensor_copy(ysb, ps)
                                nc.gpsimd.dma_start(
                                    out=y_d[128 * tau:128 * (tau + 1), :],
                                    in_=ysb)

                        emit_fin(h, 2)
                        emit_proj(0)
                        for ki in range(8):
                            emit_av(h, 3, ki, atts_prev)
                        emit_proj(1)
                        for ki in range(8, 16):
                            emit_av(h, 3, ki, atts_prev)
                        emit_den(h, 3)
                        emit_proj(2)
                        emit_fin(h, 3)
                        emit_proj(3)

    nc.compile()
    return nc


def _prep_inputs(x, w_qkv, b_qkv, w_out, b_out):
    x = np.asarray(x, np.float32)
    w_qkv = np.asarray(w_qkv, np.float32)
    b_qkv = np.asarray(b_qkv, np.float32)
    w_out = np.asarray(w_out, np.float32)
    bf = ml_dtypes.bfloat16
    in_maps = []
    for c in range(8):
        b, g = c // 2, c % 2
        xt = np.ascontiguousarray(x[b].T)
        wqk = np.concatenate(
            [w_qkv[:, 512 * g:512 * g + 512],
             w_qkv[:, C + 512 * g:C + 512 * g + 512]], axis=1)
        bqk = np.concatenate(
            [b_qkv[512 * g:512 * g + 512],
             b_qkv[C + 512 * g:C + 512 * g + 512]]).reshape(8, 128).T
        wv = w_qkv[:, 2 * C + 512 * g:2 * C + 512 * g + 512]
        wo = w_out[512 * g:512 * g + 512, :]
        in_maps.append({
            "xt": np.ascontiguousarray(xt.astype(bf)),
            "wqk": np.ascontiguousarray(wqk.astype(bf)),
            "bqk": np.ascontiguousarray(bqk),
            "wv": np.ascontiguousarray(wv.astype(bf)),
            "wo": np.ascontiguousarray(wo.astype(bf)),
        })
    return in_maps


def kernel(x, w_qkv, b_qkv, w_out, b_out):
    if "nc" not in _cache:
        _cache["nc"] = _build()
    nc = _cache["nc"]
    in_maps = _prep_inputs(x, w_qkv, b_qkv, w_out, b_out)
    res = run_bass_kernel_spmd(nc, in_maps, list(range(8)))
    b_out = np.asarray(b_out, np.float32)
    b_qkv = np.asarray(b_qkv, np.float32)
    w_out = np.asarray(w_out, np.float32)
    # v-bias folded here: softmax rows sum to 1, so +b_v shifts the
    # attention output by b_v, contributing b_v @ w_out to y
    bias = b_out + b_qkv[2 * C:3 * C] @ w_out
    out = np.empty((B, T, C), np.float32)
    for b in range(B):
        out[b] = (res.results[2 * b]["y"].astype(np.float32)
                  + res.results[2 * b + 1]["y"].astype(np.float32) + bias)
    return out


def bench(x, w_qkv, b_qkv, w_out, b_out, iters=16, reps=3, loop=None, phases=3):
    """Time the NEFF on hardware. The kernel body is wrapped in a For_i
    hardware loop of `iters` iterations (one dispatch); subtracting the
    1-iteration dispatch time cancels network/dispatch overhead.
    Returns per-execution seconds."""
    import time
    import jax
    import jax.numpy as jnp
    from jax.sharding import Mesh, PartitionSpec
    from jax.experimental.shard_map import shard_map
    from concourse import bass2jax
    from concourse.bass2jax import (
        _bass_exec_p, install_neuronx_cc_hook, partition_id_tensor)

    if (loop is not None and loop > 1) or phases != 3:
        nc = _build(loop=loop or 1, phases=phases)
    else:
        nc = _cache.setdefault("nc", _build())
    install_neuronx_cc_hook()
    in_maps = _prep_inputs(x, w_qkv, b_qkv, w_out, b_out)

    partition_name = (nc.partition_id_tensor.name
                      if nc.partition_id_tensor else None)
    in_names, out_names, out_avals, zero_outs = [], [], [], []
    for alloc in nc.m.functions[0].allocations:
        if not isinstance(alloc, mybir.MemoryLocationSet):
            continue
        name = alloc.memorylocations[0].name
        if alloc.kind == "ExternalInput":
            if name != partition_name:
                in_names.append(name)
        elif alloc.kind == "ExternalOutput":
            out_names.append(name)
            shape = tuple(alloc.tensor_shape)
            dtype = mybir.dt.np(alloc.dtype)
            out_avals.append(jax.core.ShapedArray(shape, dtype))
            zero_outs.append(np.zeros(shape, dtype))
    n_params = len(in_names)
    all_names = in_names + out_names
    if partition_name is not None:
        all_names.append(partition_name)
    chain_idx = in_names.index("bqk")

    def body_n(n):
        def _body(*args):
            ins = list(args)
            outs = None
            for _ in range(n):
                cur = list(ins)
                if outs is not None:
                    y = outs[0]
                    cur[chain_idx] = cur[chain_idx] + 0.0 * y[:128, :8]
                if partition_name is not None:
                    cur.append(partition_id_tensor())
                outs = _bass_exec_p.bind(
                    *cur,
                    out_avals=tuple(out_avals),
                    in_names=tuple(all_names),
                    out_names=tuple(out_names),
                    lowering_input_output_aliases=(),
                    sim_require_finite=True,
                    sim_require_nnan=True,
                    nc=nc,
                )
            return tuple(outs)
        return _body

    devices = jax.devices()[:8]
    mesh = Mesh(np.asarray(devices), ("core",))
    in_specs = (PartitionSpec("core"),) * (n_params + len(out_names))
    out_specs = (PartitionSpec("core"),) * len(out_names)

    per_core = [[np.asarray(m[name]) for name in in_names] for m in in_maps]
    concat_in = [np.concatenate([per_core[c][i] for c in range(8)], axis=0)
                 for i in range(n_params)]
    concat_zero = [np.zeros((8 * z.shape[0], *z.shape[1:]), z.dtype)
                   for z in zero_outs]
    ins_dev = [jax.device_put(a) for a in concat_in]
    donate = tuple(range(n_params, n_params + len(zero_outs)))

    f = jax.jit(shard_map(body_n(1), mesh=mesh, in_specs=in_specs,
                          out_specs=out_specs, check_rep=False),
                donate_argnums=donate, keep_unused=True)

    def fresh_zeros(n):
        return [[jax.device_put(z) for z in concat_zero] for _ in range(n)]

    z0 = fresh_zeros(1)[0]
    jax.block_until_ready(f(*ins_dev, *z0))  # compile + warm

    def timed():
        best = float("inf")
        for _ in range(reps):
            zs = fresh_zeros(1)[0]
            jax.block_until_ready(zs)
            t0 = time.perf_counter()
            r = f(*ins_dev, *zs)
            jax.block_until_ready(r)
            best = min(best, time.perf_counter() - t0)
        return best

    return timed()


# revision 29
# speedup vs baseline: 5.3792x; 1.4854x over previous
"""Causal self-attention (B=4, T=2048, C=1024, H=16) on 8 trn2 NeuronCores.

Sharding: core c -> (batch b = c//2, head-group g = c%2 of 8 heads).
Each core computes its batch's QKV for its 8 heads, causal attention,
and a partial output projection (its heads' rows of w_out). Host sums
the two partials per batch and adds b_out.

v2 design (vs v1 baseline):
- Weights (wqk/wv/wo/bqk) are DMA'd into SBUF once, OUTSIDE the For_i
  loop: kills the ~20us PE stall at each iteration start.
- All matmuls in bf16 (x shipped as bf16): no fp32r small-free-dim
  penalty, half the DMA, half the SBUF.
- The gpsimd affine_select causal mask is replaced by a DVE multiply
  with a precomputed [128,128] mask tile.
- One fused PE instruction stream instead of 3 phases: V for all
  tokens + QK for head-pair 0 first, then 8 per-head "streams"
  (scores+exp for head h) with PE filler work interleaved under the
  ACT exp latency: AV matmuls of head h-1 and QK matmuls of the next
  head pair. Projection drains at the end, overlapped with AV of the
  last head. This keeps the PE dense so the HAM clock-gate stays at
  2.4 GHz (it throttles to 1.2 GHz when the PE has idle windows).
"""
import sys
sys.path.insert(0, "/opt/trn_rl_repo")

from collections import deque

import numpy as np
import ml_dtypes
import concourse.bass as bass
import concourse.mybir as mybir
import concourse.tile as tile
from concourse import bacc
from concourse.bass_utils import run_bass_kernel_spmd
from concourse.tile import TileContext

F32 = mybir.dt.float32
BF16 = mybir.dt.bfloat16
AF = mybir.ActivationFunctionType

B, T, C = 4, 2048, 1024
H, D = 16, 64
HL = 8            # heads per core
PAIRS = HL // 2   # head pairs (128-partition stacking)
KCH = C // 128    # contraction chunks for QKV
TG = T // 512     # 512-wide token groups
NKT = T // 128    # 128-wide key tiles
SCALE = D ** -0.5

_cache = {}


def _build(loop=1, phases=3):
    from contextlib import nullcontext
    nc = bacc.Bacc("TRN2", target_bir_lowering=False, debug=False, num_devices=8)

    xt_d = nc.dram_tensor("xt", [C, T], BF16, kind="ExternalInput")
    wqk_d = nc.dram_tensor("wqk", [C, 1024], BF16, kind="ExternalInput")
    wv_d = nc.dram_tensor("wv", [C, 512], BF16, kind="ExternalInput")
    bqk_d = nc.dram_tensor("bqk", [128, 8], F32, kind="ExternalInput")
    wo_d = nc.dram_tensor("wo", [512, 1024], BF16, kind="ExternalInput")
    y_d = nc.dram_tensor("y", [T, C], BF16, kind="ExternalOutput")

    with TileContext(nc) as tc:
        with tc.tile_pool(name="persist", bufs=1) as persist:
            wqk_sb = persist.tile([128, KCH, 1024], BF16)
            wo_sb = persist.tile([128, PAIRS, 1024], BF16)
            bqk_sb = persist.tile([128, 8], F32)
            xall = persist.tile([128, KCH, T], BF16)
            qkT = persist.tile([128, 8, T], BF16)       # t 0-3: q pairs, 4-7: k pairs
            v_aug = persist.tile([128, NKT, HL, D + 1], BF16)
            attout = persist.tile([128, PAIRS, T], BF16)
            mask = persist.tile([128, 128], BF16)       # causal: 1 if q>=k else 0
            ones_bf = persist.tile([1, 128], BF16)

            # weights + constants: once, outside the loop
            nc.sync.dma_start(out=bqk_sb, in_=bqk_d[:])
            nc.sync.dma_start(
                out=wqk_sb, in_=wqk_d.rearrange("(k p) c -> p k c", p=128))
            nc.sync.dma_start(
                out=wo_sb, in_=wo_d.rearrange("(p c) e -> c p e", c=128))
            nc.vector.memset(ones_bf, 1.0)
            nc.vector.memset(mask, 1.0)
            nc.gpsimd.affine_select(
                out=mask, in_=mask,
                compare_op=mybir.AluOpType.is_ge, fill=0.0,
                base=0, pattern=[[1, 128]], channel_multiplier=-1)
            nc.vector.memset(v_aug[:, :, :, D:D + 1], 1.0)

            xt_r = xt_d.rearrange("(k p) t -> p k t", p=128)

            def body():
                with tc.tile_pool(name="qk_ps", bufs=2, space="PSUM") as qk_psum, \
                     tc.tile_pool(name="av_ps", bufs=2, space="PSUM") as av_psum, \
                     tc.tile_pool(name="att", bufs=2) as att_pool, \
                     tc.tile_pool(name="nrm", bufs=2) as nrm_pool:

                    # x for the whole iteration (per-group DMAs, sync queue)
                    for g in range(TG):
                        nc.sync.dma_start(
                            out=xall[:, :, 512 * g:512 * (g + 1)],
                            in_=xt_r[:, :, 512 * g:512 * (g + 1)])

                    def emit_qk(t, g):
                        # q or k feature tile t for 512-token group g
                        ps = qk_psum.tile([128, 512], F32, tag="qk")
                        for k in range(KCH):
                            nc.tensor.matmul(
                                ps, wqk_sb[:, k, 128 * t:128 * (t + 1)],
                                xall[:, k, 512 * g:512 * (g + 1)],
                                start=(k == 0), stop=(k == KCH - 1))
                        nc.vector.tensor_scalar_add(
                            qkT[:, t, 512 * g:512 * (g + 1)], ps,
                            bqk_sb[:, t:t + 1])

                    # ---- stage A: V for all tokens + QK for pair 0 ----
                    # V bias is folded into the host-side output bias
                    # (softmax weights sum to 1, so +b_v adds b_v @ w_out to y)
                    with tc.tile_pool(name="v_ps", bufs=2, space="PSUM") as v_psum, \
                         tc.tile_pool(name="wvp", bufs=1) as wv_pool:
                        wv_sb = wv_pool.tile([128, KCH, 512], BF16, tag="wv")
                        nc.sync.dma_start(
                            out=wv_sb,
                            in_=wv_d.rearrange("(k p) c -> p k c", p=128))

                        def emit_v(g, tt):
                            tau = 4 * g + tt
                            ps = v_psum.tile([128, 512], F32, tag="v")
                            for k in range(KCH):
                                nc.tensor.matmul(
                                    ps, xall[:, k, 128 * tau:128 * (tau + 1)],
                                    wv_sb[:, k, :], start=(k == 0),
                                    stop=(k == KCH - 1))
                            nc.vector.tensor_copy(
                                v_aug[:, tau, :, 0:D],
                                ps.rearrange("p (h d) -> p h d", h=HL))

                        for g in range(TG):
                            emit_qk(0, g)
                            emit_v(g, 0)
                            emit_v(g, 1)
                            emit_qk(4, g)
                            emit_v(g, 2)
                            emit_v(g, 3)

                    # ---- filler units (run on PE under the ACT exp) ----
                    def emit_av(h, gi, ki, atts):
                        p, r = h // 2, 64 * (h % 2)
                        qlo = 128 * ki
                        g0 = 512 * gi
                        lo = max(g0, qlo)
                        av = _av_state.get((h, gi))
                        if av is None:
                            av = av_psum.tile([65, 512], F32, tag="av")
                            _av_state[(h, gi)] = av
                        nc.tensor.matmul(
                            av[:, lo - g0:512],
                            v_aug[:, ki, h, :],
                            atts[ki][:, lo - qlo:512 * (gi + 1) - qlo],
                            start=(ki == 0), stop=(ki == 4 * gi + 3))

                    def emit_den(h, gi):
                        av = _av_state[(h, gi)]
                        den = nrm_pool.tile([1, 512], F32, tag="den")
                        nc.vector.tensor_copy(den, av[64:65, :])
                        nc.vector.reciprocal_approx_fast(den, den)
                        _den_state[(h, gi)] = den

                    def emit_fin(h, gi):
                        p, r = h // 2, 64 * (h % 2)
                        av = _av_state.pop((h, gi))
                        rden = _den_state.pop((h, gi))
                        rbc = nrm_pool.tile([64, 512], F32, tag="rbc")
                        nc.gpsimd.partition_broadcast(rbc, rden)
                        nc.vector.tensor_mul(
                            attout[r:r + 64, p, 512 * gi:512 * (gi + 1)],
                            av[0:64, :], rbc)

                    _av_state = {}
                    _den_state = {}

                    UNIT_COST = {"av": 512, "den": 64, "fin": 64, "qk": 8 * 512}

                    def build_units(lag_h, lag_atts, qk_ts):
                        units = []
                        if lag_h is not None:
                            for gi in range(TG):
                                for ki in range(4 * gi + 4):
                                    units.append(("av", lag_h, gi, ki, lag_atts))
                                units.append(("den", lag_h, gi))
                                units.append(("fin", lag_h, gi))
                        # stagger each fin 2 units later so the DVE mul does
                        # not wait back-to-back on the den/recip/bcast chain
                        out = []
                        pending = deque()
                        for u in units:
                            if u[0] == "fin":
                                pending.append((len(out) + 2, u))
                            else:
                                out.append(u)
                            while pending and pending[0][0] <= len(out):
                                out.insert(pending[0][0], pending.popleft()[1])
                        while pending:
                            out.append(pending.popleft()[1])
                        # spread the qk groups through the unit list
                        qk_units = [("qk", t, j) for t in qk_ts for j in range(TG)]
                        if qk_units:
                            n = len(qk_units)
                            step = max(1, len(out) // n)
                            for j, u in enumerate(qk_units):
                                out.insert(min(len(out), (j + 1) * step + j), u)
                        return out

                    def run_unit(u):
                        if u[0] == "av":
                            emit_av(u[1], u[2], u[3], u[4])
                        elif u[0] == "den":
                            emit_den(u[1], u[2])
                        elif u[0] == "fin":
                            emit_fin(u[1], u[2])
                        elif u[0] == "qk":
                            emit_qk(u[1], u[2])

                    # ---- per-head streams ----
                    # sc is double-buffered as 2x [128,1024] (2 banks each):
                    # the score matmuls of part j+1 overlap the exp of part j,
                    # so the PE never waits a full exp per key tile.
                    with tc.tile_pool(name="sc_ps", bufs=2, space="PSUM") as sc_psum:
                        atts_prev = None
                        qk_sched = [[1, 5], [2], [6], [3], [7], [], [], []]
                        for h in range(HL):
                            p, r = h // 2, 64 * (h % 2)
                            q_t = qkT[r:r + 64, p, :]
                            k_t = qkT[r:r + 64, 4 + p, :]
                            units = deque(build_units(
                                h - 1 if h >= 1 else None, atts_prev,
                                qk_sched[h]))
                            total_cost = sum(UNIT_COST[u[0]] for u in units)
                            exp_total = sum(T - 128 * ki for ki in range(NKT))
                            atts_cur = []
                            for ki in range(NKT):
                                qlo = 128 * ki
                                w = T - qlo
                                at = att_pool.tile([128, w], BF16,
                                                   tag=f"att{ki}")
                                lhsT = k_t[:, qlo:qlo + 128]
                                parts = [(qlo, min(qlo + 1024, T))]
                                if w > 1024:
                                    parts.append((qlo + 1024, T))
                                for plo, phi in parts:
                                    sc = sc_psum.tile([128, 1024], F32,
                                                      tag="sc")
                                    # 512-chunks relative to the tile base so
                                    # no matmul output crosses a PSUM bank
                                    qc = plo
                                    while qc < phi:
                                        qe = min(qc + 512, phi)
                                        nc.tensor.matmul(
                                            sc[:, qc - plo:qe - plo], lhsT,
                                            q_t[:, qc:qe],
                                            start=True, stop=True)
                                        qc = qe
                                    nc.scalar.activation(
                                        at[:, plo - qlo:phi - qlo],
                                        sc[:, 0:phi - plo], AF.Exp,
                                        scale=SCALE)
                                    if plo == qlo:
                                        nc.vector.tensor_mul(
                                            at[:, 0:128], at[:, 0:128], mask)
                                    # consume fillers proportional to exp size
                                    budget = (phi - plo) * total_cost / exp_total
                                    while units and budget > 0:
                                        u = units.popleft()
                                        run_unit(u)
                                        budget -= UNIT_COST[u[0]]
                                atts_cur.append(at)
                                # last head: AV for completed gi-groups runs
                                # in-stream (no next stream to lag into)
                                if h == HL - 1 and ki % 4 == 3 and ki < 12:
                                    gi = ki // 4
                                    for kj in range(4 * gi + 4):
                                        emit_av(h, gi, kj, atts_cur)
                                    emit_den(h, gi)
                                    if gi >= 1:
                                        emit_fin(h, gi - 1)
                            while units:
                                run_unit(units.popleft())
                            atts_prev = atts_cur

                    # ---- drain: last AV group of head 7 + projection ----
                    with tc.tile_pool(name="y_ps", bufs=2, space="PSUM") as y_psum, \
                         tc.tile_pool(name="ysb", bufs=3) as ypool:
                        h = HL - 1

                        def emit_proj(gi):
                            for tt in range(4):
                                tau = 4 * gi + tt
                                ps = y_psum.tile([128, 1024], F32, tag="y")
                                for pp in range(PAIRS):
                                    nc.tensor.matmul(
                                        ps[:, 0:512],
                                        attout[:, pp, 128 * tau:128 * (tau + 1)],
                                        wo_sb[:, pp, 0:512],
                                        start=(pp == 0), stop=(pp == PAIRS - 1))
                                for pp in range(PAIRS):
                                    nc.tensor.matmul(
                                        ps[:, 512:1024],
                                        attout[:, pp, 128 * tau:128 * (tau + 1)],
                                        wo_sb[:, pp, 512:1024],
                                        start=(pp == 0), stop=(pp == PAIRS - 1))
                                ysb = ypool.tile([128, 1024], BF16, tag="ysb")
                                nc.vector.tensor_copy(ysb, ps)
                                nc.gpsimd.dma_start(
                                    out=y_d[128 * tau:128 * (tau + 1), :],
                                    in_=ysb)

                        emit_fin(h, 2)
                        emit_proj(0)
                        for ki in range(8):
                            emit_av(h, 3, ki, atts_prev)
                        emit_proj(1)
                        for ki in range(8, 16):
                            emit_av(h, 3, ki, atts_prev)
                        emit_den(h, 3)
                        emit_proj(2)
                        emit_fin(h, 3)
                        emit_proj(3)

            if loop > 1:
                # unroll bodies inside For_i: the hardware loop carries an
                # all-engine barrier per trip, so fewer trips = less overhead
                n_un = 4 if loop % 4 == 0 else (2 if loop % 2 == 0 else 1)
                with tc.For_i(0, loop // n_un, 1):
                    for _ in range(n_un):
                        body()
            else:
                body()

    nc.compile()
    return nc


def _prep_inputs(x, w_qkv, b_qkv, w_out, b_out):
    x = np.asarray(x, np.float32)
    w_qkv = np.asarray(w_qkv, np.float32)
    b_qkv = np.asarray(b_qkv, np.float32)
    w_out = np.asarray(w_out, np.float32)
    bf = ml_dtypes.bfloat16
    in_maps = []
    for c in range(8):
        b, g = c // 2, c % 2
        xt = np.ascontiguousarray(x[b].T)
        wqk = np.concatenate(
            [w_qkv[:, 512 * g:512 * g + 512],
             w_qkv[:, C + 512 * g:C + 512 * g + 512]], axis=1)
        bqk = np.concatenate(
            [b_qkv[512 * g:512 * g + 512],
             b_qkv[C + 512 * g:C + 512 * g + 512]]).reshape(8, 128).T
        wv = w_qkv[:, 2 * C + 512 * g:2 * C + 512 * g + 512]
        wo = w_out[512 * g:512 * g + 512, :]
        in_maps.append({
            "xt": np.ascontiguousarray(xt.astype(bf)),
            "wqk": np.ascontiguousarray(wqk.astype(bf)),
            "bqk": np.ascontiguousarray(bqk),
            "wv": np.ascontiguousarray(wv.astype(bf)),
            "wo": np.ascontiguousarray(wo.astype(bf)),
        })
    return in_maps


def kernel(x, w_qkv, b_qkv, w_out, b_out):
    if "nc" not in _cache:
        _cache["nc"] = _build()
    nc = _cache["nc"]
    in_maps = _prep_inputs(x, w_qkv, b_qkv, w_out, b_out)
    res = run_bass_kernel_spmd(nc, in_maps, list(range(8)))
    b_out = np.asarray(b_out, np.float32)
    b_qkv = np.asarray(b_qkv, np.float32)
    w_out = np.asarray(w_out, np.float32)
    # v-bias folded here: softmax rows sum to 1, so +b_v shifts the
    # attention output by b_v, contributing b_v @ w_out to y
    bias = b_out + b_qkv[2 * C:3 * C] @ w_out
    out = np.empty((B, T, C), np.float32)
    for b in range(B):
        out[b] = (res.results[2 * b]["y"].astype(np.float32)
                  + res.results[2 * b + 1]["y"].astype(np.float32) + bias)
    return out


def bench(x, w_qkv, b_qkv, w_out, b_out, iters=16, reps=3, loop=None, phases=3):
    """Time the NEFF on hardware. The kernel body is wrapped in a For_i
    hardware loop of `iters` iterations (one dispatch); subtracting the
    1-iteration dispatch time cancels network/dispatch overhead.
    Returns per-execution seconds."""
    import time
    import jax
    import jax.numpy as jnp
    from jax.sharding import Mesh, PartitionSpec
    from jax.experimental.shard_map import shard_map
    from concourse import bass2jax
    from concourse.bass2jax import (
        _bass_exec_p, install_neuronx_cc_hook, partition_id_tensor)

    if (loop is not None and loop > 1) or phases != 3:
        nc = _build(loop=loop or 1, phases=phases)
    else:
        nc = _cache.setdefault("nc", _build())
    install_neuronx_cc_hook()
    in_maps = _prep_inputs(x, w_qkv, b_qkv, w_out, b_out)

    partition_name = (nc.partition_id_tensor.name
                      if nc.partition_id_tensor else None)
    in_names, out_names, out_avals, zero_outs = [], [], [], []
    for alloc in nc.m.functions[0].allocations:
        if not isinstance(alloc, mybir.MemoryLocationSet):
            continue
        name = alloc.memorylocations[0].name
        if alloc.kind == "ExternalInput":
            if name != partition_name:
                in_names.append(name)
        elif alloc.kind == "ExternalOutput":
            out_names.append(name)
            shape = tuple(alloc.tensor_shape)
            dtype = mybir.dt.np(alloc.dtype)
            out_avals.append(jax.core.ShapedArray(shape, dtype))
            zero_outs.append(np.zeros(shape, dtype))
    n_params = len(in_names)
    all_names = in_names + out_names
    if partition_name is not None:
        all_names.append(partition_name)
    chain_idx = in_names.index("bqk")

    def body_n(n):
        def _body(*args):
            ins = list(args)
            outs = None
            for _ in range(n):
                cur = list(ins)
                if outs is not None:
                    y = outs[0]
                    cur[chain_idx] = cur[chain_idx] + 0.0 * y[:128, :8]
                if partition_name is not None:
                    cur.append(partition_id_tensor())
                outs = _bass_exec_p.bind(
                    *cur,
                    out_avals=tuple(out_avals),
                    in_names=tuple(all_names),
                    out_names=tuple(out_names),
                    lowering_input_output_aliases=(),
                    sim_require_finite=True,
                    sim_require_nnan=True,
                    nc=nc,
                )
            return tuple(outs)
        return _body

    devices = jax.devices()[:8]
    mesh = Mesh(np.asarray(devices), ("core",))
    in_specs = (PartitionSpec("core"),) * (n_params + len(out_names))
    out_specs = (PartitionSpec("core"),) * len(out_names)

    per_core = [[np.asarray(m[name]) for name in in_names] for m in in_maps]
    concat_in = [np.concatenate([per_core[c][i] for c in range(8)], axis=0)
                 for i in range(n_params)]
    concat_zero = [np.zeros((8 * z.shape[0], *z.shape[1:]), z.dtype)
                   for z in zero_outs]
    ins_dev = [jax.device_put(a) for a in concat_in]
    donate = tuple(range(n_params, n_params + len(zero_outs)))

    f = jax.jit(shard_map(body_n(1), mesh=mesh, in_specs=in_specs,
                          out_specs=out_specs, check_rep=False),
                donate_argnums=donate, keep_unused=True)

    def fresh_zeros(n):
        return [[jax.device_put(z) for z in concat_zero] for _ in range(n)]

    z0 = fresh_zeros(1)[0]
    jax.block_until_ready(f(*ins_dev, *z0))  # compile + warm

    def timed():
        best = float("inf")
        for _ in range(reps):
            zs = fresh_zeros(1)[0]
            jax.block_until_ready(zs)
            t0 = time.perf_counter()
            r = f(*ins_dev, *zs)
            jax.block_until_ready(r)
            best = min(best, time.perf_counter() - t0)
        return best

    return timed()
